# revision 1
# baseline (speedup 1.0000x reference)
"""Trainium2 Bass kernel for nn_DeChunkLayer (H-Net dechunk: EMA over chunks +
broadcast back to token positions).

Formulation: instead of (argsort -> EMA over M -> gather back to L), run ONE
first-order linear recurrence over the L-length token axis:
    a_l = mask_l ? (1 - p_l) : 1
    b_l = mask_l ? p_l * x[pbi_l] : 0        (pbi = cumsum(mask) - 1)
    H_l = a_l * H_{l-1} + b_l
Then out[l] = H_l exactly (at the m-th boundary H becomes ema[m]; in between it
holds). No argsort/compaction and no output gather; the only data-dependent
movement is the row gather x[pbi_l], done with the HW-accelerated dma_gather.

Per chunk of 128 positions the recurrence is solved with matmuls:
    out[i] = sum_{j<=i} exp(S_i - S_j + ln s_j) * x[pbi_j]  +  exp(S_i) * H_prev
(S = within-chunk inclusive cumsum of log a; ln s folds the s_j scale and the
boundary mask into the exp bias). The chunk is laid out REVERSED on the output
partitions (row 0 = chunk end), which makes the inter-chunk state h_c EQUAL
the chunk's final output row 0: the carry rank-1 matmul for chunk m reads its
rhs directly from row 0 of chunk m-1's staged output tile in SBUF -- there is
no separate H-chain op at all.

Performance structure (sim: 320us baseline -> 117us):
 - bf16 data path (tolerance is 2e-2): x ships bf16 from the host, the gather
   moves 2KB rows, the Exp activation emits bf16 weights, and the output is
   written bf16 and upconverted on the host. fp8 x was tried and FAILS the
   tolerance (e3m4's 3.1% relerr lands on max-magnitude outputs).
 - The triangular -inf mask is preloaded into PSUM via a transpose-matmul of
   the (symmetric) mask matrix, so weight build is PE+ACT only.
 - All 64 carry rows (esr) are computed in ONE batched setup exp and bounced
   through DRAM into a single partition-0 row (matmul lhsT must sit at
   partition base 0).
 - Software pipelining: weights for chunk c+1 and the gather for chunk c+PF
   are issued before chunk c's dependent tail; chunk m's carry+copies+DMA are
   issued one iteration late so no engine queue head-of-line blocks the
   serial carry[q] -> copy[q] -> carry[q] path. Carries and PSUM->SBUF copies
   run at aligned quarter granularity; DVE and ACT split the quarters
   ~950ns/chunk each, with output DMA flushed per half.
 - GPSIMD (Pool) cannot touch PSUM; it only runs gather descriptor gen,
   batched GB=2 chunks per call (994ns fixed SWDGE overhead per call) and
   interleaved finely with the output DMAs on the shared DMA engines.
 - Setup: the gather-index pipeline (wrapped-16 cumsum -> int16 idx) runs
   first, in fp16 (values <= 2048 exact), from one packed input DMA, so the
   first gather fires ~6us in while the weights path streams in behind.

Sharded over batch: core b handles batch row b.
"""

import numpy as np

import concourse.bass as bass
import concourse.tile as tile
from concourse import bacc, mybir
from concourse.bass import IndirectOffsetOnAxis

F32 = mybir.dt.float32
BF16 = mybir.dt.bfloat16
FP16 = mybir.dt.float16
I16 = mybir.dt.int16
I32 = mybir.dt.int32
AX = mybir.AluOpType
ACT = mybir.ActivationFunctionType

# Problem constants (hardcoded per contract)
B, L, D, M = 8, 8192, 1024, 2048
EPS = 1e-4
N_CORES = 8
NEG_BIG = -1e30


def build_program(L_=L, D_=D, M_=M, reps=1, gather_mode="antgather"):
    """Build the per-core Bass program. Returns (nc, names dict)."""
    CH = 128                       # chunk length (= matmul K)
    NCH = L_ // CH                 # number of chunks
    NF = L_ // 16                  # wrapped-16 index columns
    assert NCH * CH == L_
    NSPL = min(512, D_)            # matmul free-dim split (psum bank = 512 f32)
    NH = D_ // NSPL

    from contextlib import ExitStack

    nc = bacc.Bacc(None, target_bir_lowering=False, debug=False)
    with tile.TileContext(nc) as tc, ExitStack() as ctx:
        dram = ctx.enter_context(tc.tile_pool(name="dram", bufs=1, space="DRAM"))
        x_d = dram.tile([M_, D_], BF16, kind="ExternalInput")
        # packed setup inputs: one DMA each (HWDGE gen is per-partition-count)
        s16_d = dram.tile([16, 160 + NF], FP16, kind="ExternalInput")
        big_d = dram.tile([128, 384], F32, kind="ExternalInput")
        pm_d = dram.tile([NCH, 2 * CH], F32, kind="ExternalInput")
        out_d = dram.tile([L_, D_], BF16, kind="ExternalOutput")

        setup = ctx.enter_context(tc.tile_pool(name="setup", bufs=1))
        bsp = ctx.enter_context(tc.tile_pool(name="bsp", bufs=2, space="PSUM"))
        xgp = ctx.enter_context(tc.tile_pool(name="xgp", bufs=3))
        ttp = ctx.enter_context(tc.tile_pool(name="ttp", bufs=3))
        osb = ctx.enter_context(tc.tile_pool(name="osb", bufs=10))

        # ---------------- setup ----------------
        # The gather-index pipeline (m16 -> c16 -> idx16) loads and computes
        # FIRST (in fp16: all values <= 2048 are exact) so the first
        # dma_gather starts while the rest of setup streams in behind it.
        s16 = setup.tile([16, 160 + NF], FP16)
        nc.sync.dma_start(out=s16[:], in_=s16_d[:])
        le16 = s16[:, 0:16]
        gt16 = s16[:, 16:32]
        rep16 = s16[:, 32:160]
        m16 = s16[:, 160:160 + NF]
        ones16 = setup.tile([16, 1], FP16)
        nc.vector.memset(ones16[:], 1.0)

        # FAST PATH: the first PF gathers need only the first 32 wrapped
        # columns; compute those in a separate small tile so the first
        # gather fires ~6us earlier (per-partition cumsum is column-local)
        FW = 32
        idx16a = setup.tile([128, FW], I16)
        idx16b = setup.tile([128, NF - FW], I16)
        with tc.tile_pool(name="bsps", bufs=1, space="PSUM") as bsps:
            cm = setup.tile([16, FW], FP16)
            nc.vector.tensor_tensor_scan(
                out=cm[:], data0=ones16[:].to_broadcast([16, FW]),
                data1=m16[:, 0:FW],
                initial=0.0, op0=AX.mult, op1=AX.add)
            mini_ps = bsps.tile([16, FW], F32, tag="bsm")
            nc.tensor.matmul(out=mini_ps[0:16, 0:FW], lhsT=le16,
                             rhs=cm[:], start=True, stop=False,
                             skip_group_check=True)
            nc.tensor.matmul(out=mini_ps[0:16, 1:FW], lhsT=gt16,
                             rhs=cm[0:16, 0:FW - 1],
                             start=False, stop=True, skip_group_check=True)
            pbi_m = setup.tile([16, FW], FP16)
            nc.vector.tensor_scalar_add(out=pbi_m[:],
                                        in0=mini_ps[0:16, 0:FW],
                                        scalar1=-1.0)
            minir_ps = bsps.tile([128, FW], F32, tag="bsmr")
            nc.tensor.matmul(out=minir_ps[0:128, 0:FW], lhsT=rep16,
                             rhs=pbi_m[:], start=True, stop=True)
            nc.vector.tensor_copy(out=idx16a[:],
                                  in_=minir_ps[0:128, 0:FW])

            # full-width path for the remaining columns
            c16 = setup.tile([16, NF], FP16)
            nc.vector.tensor_tensor_scan(
                out=c16[:], data0=ones16[:].to_broadcast([16, NF]),
                data1=m16,
                initial=0.0, op0=AX.mult, op1=AX.add)
            pbi16_ps = bsps.tile([16, NF], F32, tag="bs16")
            nc.tensor.matmul(out=pbi16_ps[0:16, FW:NF], lhsT=le16,
                             rhs=c16[:, FW:NF], start=True, stop=False,
                             skip_group_check=True)
            nc.tensor.matmul(out=pbi16_ps[0:16, FW:NF], lhsT=gt16,
                             rhs=c16[0:16, FW - 1:NF - 1],
                             start=False, stop=True, skip_group_check=True)
            pbi16 = setup.tile([16, NF], FP16)
            nc.vector.tensor_scalar_add(out=pbi16[0:16, FW:NF],
                                        in0=pbi16_ps[0:16, FW:NF],
                                        scalar1=-1.0)
            # replicate the 16 wrapped index rows to all 8 gpsimd core
            # slots with one fp16 matmul (values <= 2047: exact)
            idxrep_ps = bsps.tile([128, NF], F32, tag="bs16r")
            nc.tensor.matmul(out=idxrep_ps[0:128, FW:NF], lhsT=rep16,
                             rhs=pbi16[:, FW:NF], start=True, stop=True)
            nc.vector.tensor_copy(out=idx16b[:],
                                  in_=idxrep_ps[0:128, FW:NF])

        big = setup.tile([128, 384], F32)
        nc.sync.dma_start(out=big[:], in_=big_d[:])
        ident = big[:, 0:128]
        rev128 = big[:, 128:256]
        mnegr = big[:, 256:384]

        pm = setup.tile([NCH, 2 * CH], F32)
        nc.sync.dma_start(out=pm[:], in_=pm_d[:])
        praw = pm[:, 0:CH]
        mk = pm[:, CH:2 * CH]

        ones_r = setup.tile([NCH, CH], F32)
        nc.vector.memset(ones_r[:], 1.0)

        pc = setup.tile([NCH, CH], F32)
        nc.vector.tensor_scalar(out=pc[:], in0=praw, scalar1=EPS,
                                scalar2=1.0 - EPS, op0=AX.max, op1=AX.min)
        q = setup.tile([NCH, CH], F32)
        nc.vector.tensor_scalar(out=q[:], in0=pc[:], scalar1=-1.0,
                                scalar2=1.0, op0=AX.mult, op1=AX.add)
        lnq = setup.tile([NCH, CH], F32)
        nc.scalar.activation(out=lnq[:], in_=q[:], func=ACT.Ln)
        loga = setup.tile([NCH, CH], F32)
        nc.vector.tensor_tensor(out=loga[:], in0=lnq[:], in1=mk, op=AX.mult)

        # ln(s) with the mask folded in: ln(p) where mask else -1e30
        lnp = setup.tile([NCH, CH], F32)
        nc.scalar.activation(out=lnp[:], in_=pc[:], func=ACT.Ln)
        lnp_m = setup.tile([NCH, CH], F32)
        nc.vector.tensor_tensor(out=lnp_m[:], in0=lnp[:], in1=mk, op=AX.mult)
        mgate = setup.tile([NCH, CH], F32)
        nc.vector.tensor_scalar(out=mgate[:], in0=mk, scalar1=-NEG_BIG,
                                scalar2=NEG_BIG, op0=AX.mult, op1=AX.add)
        lns = setup.tile([NCH, CH], F32)
        nc.vector.tensor_tensor(out=lns[:], in0=lnp_m[:], in1=mgate[:],
                                op=AX.add)

        # within-chunk inclusive cumsum of log(a) (along free dim)
        s_i = setup.tile([NCH, CH], F32)
        nc.vector.tensor_tensor_scan(out=s_i[:], data0=ones_r[:], data1=loga[:],
                                     initial=0.0, op0=AX.mult, op1=AX.add)

        # indices in [CH, NCH] int32 layout (for the indirect_dma fallback)
        c_i = setup.tile([NCH, CH], F32)
        nc.vector.tensor_tensor_scan(out=c_i[:], data0=ones_r[:], data1=mk[:],
                                     initial=0.0, op0=AX.mult, op1=AX.add)
        cnt_colT = bsp.tile([128, 128], F32, tag="bs")
        nc.tensor.transpose(out=cnt_colT[0:1, 0:NCH], in_=c_i[:, CH - 1:CH],
                            identity=big[0:NCH, 0:NCH])
        cnt_row = setup.tile([1, NCH], F32)
        nc.vector.tensor_copy(out=cnt_row[:], in_=cnt_colT[0:1, 0:NCH])
        ones1 = setup.tile([1, 128], F32)
        nc.vector.memset(ones1[:], 1.0)
        cum_row = setup.tile([1, NCH], F32)
        nc.vector.tensor_tensor_scan(out=cum_row[:], data0=ones1[0:1, 0:NCH],
                                     data1=cnt_row[:], initial=0.0,
                                     op0=AX.mult, op1=AX.add)
        bases_row = setup.tile([1, NCH], F32)
        nc.vector.memset(bases_row[:], 0.0)
        nc.vector.tensor_copy(out=bases_row[0:1, 1:NCH],
                              in_=cum_row[0:1, 0:NCH - 1])
        bases_colT = bsp.tile([128, 128], F32, tag="bs")
        nc.tensor.transpose(out=bases_colT[0:NCH, 0:1], in_=bases_row[:],
                            identity=big[0:1, 0:1])
        bases_col = setup.tile([NCH, 1], F32)
        nc.vector.tensor_copy(out=bases_col[:], in_=bases_colT[0:NCH, 0:1])
        pbi_i = setup.tile([NCH, CH], F32)
        nc.vector.tensor_scalar(out=pbi_i[:], in0=c_i[:], scalar1=bases_col[:],
                                scalar2=-1.0, op0=AX.add, op1=AX.add)
        pbiT_ps = bsp.tile([128, 128], F32, tag="bs")
        nc.tensor.transpose(out=pbiT_ps[0:CH, 0:NCH], in_=pbi_i[:],
                            identity=big[0:NCH, 0:NCH])
        idxT = setup.tile([CH, NCH], I32)
        nc.vector.tensor_copy(out=idxT[:], in_=pbiT_ps[0:CH, 0:NCH])

        # transposed per-chunk columns: S, and bias = ln(s) - S
        ST_ps = bsp.tile([128, 128], F32, tag="bs")
        nc.tensor.transpose(out=ST_ps[0:CH, 0:NCH], in_=s_i[:],
                            identity=big[0:NCH, 0:NCH])
        ST = setup.tile([CH, NCH], F32)
        nc.vector.tensor_copy(out=ST[:], in_=ST_ps[0:CH, 0:NCH])
        lnsT_ps = bsp.tile([128, 128], F32, tag="bs")
        nc.tensor.transpose(out=lnsT_ps[0:CH, 0:NCH], in_=lns[:],
                            identity=big[0:NCH, 0:NCH])
        nbT = setup.tile([CH, NCH], F32)
        nc.vector.tensor_tensor(out=nbT[:], in0=lnsT_ps[0:CH, 0:NCH],
                                in1=ST[:], op=AX.subtract)

        # all carry rows at once: esr_all[c, i'] = exp(S^c_{127-i'})
        # (ST already holds s_i transposed; multiply by rev to flip free dim).
        # Matmul lhsT must sit at partition base 0, so bounce the [NCH, CH]
        # tile through DRAM and reload it as one [1, NCH*CH] partition-0 row.
        srev_ps = bsp.tile([128, 128], F32, tag="bs")
        nc.tensor.matmul(out=srev_ps[0:NCH, 0:CH], lhsT=ST[:],
                         rhs=rev128, is_transpose=True,
                         start=True, stop=True)
        esr_all = setup.tile([NCH, CH], BF16)
        nc.scalar.activation(out=esr_all[:], in_=srev_ps[0:NCH, 0:CH],
                             func=ACT.Exp)
        esr_d = dram.tile([NCH, CH], BF16)
        nc.sync.dma_start(out=esr_d[:], in_=esr_all[:])
        esr_row = setup.tile([1, NCH * CH], BF16)
        nc.sync.dma_start(out=esr_row[:],
                          in_=esr_d[:].rearrange("a b -> (a b)"))

        # ---------------- main loop ----------------
        # created after the setup's scoped psum pool is released: 2 (bs)
        # + 3*2 (outp) = 8 banks
        outp = ctx.enter_context(tc.tile_pool(name="outp", bufs=3,
                                              space="PSUM"))
        GB = 2                      # chunks per batched gather call

        for _rep in range(reps):
            xg_tiles = {}
            wt_tiles = {}

            def issue_gather(c0):
                """One SWDGE call gathers GB chunks (994ns fixed overhead per
                call); group g of the out tile = chunk c0+g."""
                if gather_mode == "antgather":
                    xgb = xgp.tile([CH, GB, D_], BF16, tag="xg")
                    if 8 * (c0 + GB) <= FW:
                        idxs = idx16a[:, 8 * c0:8 * (c0 + GB)]
                    else:
                        idxs = idx16b[:, 8 * c0 - FW:8 * (c0 + GB) - FW]
                    nc.gpsimd.dma_gather(
                        out_ap=xgb[:],
                        in_ap=x_d[:],
                        idxs_ap=idxs,
                        num_idxs=CH * GB, num_idxs_reg=CH * GB,
                        elem_size=D_)
                    for g in range(GB):
                        xg_tiles[c0 + g] = (xgb, g)
                elif gather_mode == "indirect":
                    for g in range(GB):
                        xgt = xgp.tile([CH, 1, D_], BF16, tag="xgs")
                        nc.gpsimd.indirect_dma_start(
                            out=xgt[:, 0, :], out_offset=None, in_=x_d[:],
                            in_offset=IndirectOffsetOnAxis(
                                ap=idxT[:, c0 + g:c0 + g + 1], axis=0))
                        xg_tiles[c0 + g] = (xgt, 0)
                else:
                    raise ValueError(gather_mode)

            def issue_weights(c):
                """PSUM <- mnegr (symmetric triangular -inf mask), then
                accumulate Sbc[j, i'] = S_{127-i'} (PE transpose of the
                free-broadcast S column against the anti-diagonal perm);
                then weights ttm[j, i'] = exp(S_{127-i'} - S_j + ln s_j)."""
                sbc = bsp.tile([128, 128], F32, tag="bs")
                nc.tensor.matmul(out=sbc[0:CH, 0:CH], lhsT=mnegr,
                                 rhs=ident, is_transpose=True,
                                 start=True, stop=False, skip_group_check=True)
                nc.tensor.matmul(out=sbc[0:CH, 0:CH],
                                 lhsT=ST[:, c:c + 1].to_broadcast([CH, CH]),
                                 rhs=rev128, is_transpose=True,
                                 start=False, stop=True, skip_group_check=True)
                ttm = ttp.tile([CH, CH], BF16, tag="ttm")
                nc.scalar.activation(out=ttm[:], in_=sbc[0:CH, 0:CH],
                                     func=ACT.Exp, bias=nbT[:, c:c + 1])
                wt_tiles[c] = ttm

            op_tiles = {}
            o_tiles = {}

            NQ = D_ // 4

            def finish_chunk(m):
                """Carry accumulation + output for chunk m, issued one
                iteration late. The carry rhs is row 0 of the PREVIOUS chunk's
                staged output: with reversed rows, out row 0 = position 127 =
                H at chunk end including its own carry = h_{m-1}, already in
                SBUF bf16 -- no separate H-chain op needed. Carry matmuls and
                PSUM->SBUF copies run at aligned QUARTER granularity: each
                quarter is an independent serial sub-path
                (carry[qk] -> copy[qk] -> next carry[qk]) whose copy engine
                (DVE for even quarters, ACT for odd) never gates another
                quarter's path."""
                op_t = op_tiles.pop(m)
                o_sb = osb.tile([CH, D_], BF16, tag="osb")
                o_tiles[m] = o_sb
                prev = o_tiles.pop(m - 1, None)
                for k in range(4):
                    qk = slice(k * NQ, (k + 1) * NQ)
                    if m > 0:
                        nc.tensor.matmul(out=op_t[0:CH, qk],
                                         lhsT=esr_row[0:1,
                                                      m * CH:(m + 1) * CH],
                                         rhs=prev[0:1, qk],
                                         start=False, stop=True,
                                         skip_group_check=True)
                    # DVE: q0, q2 and odd-chunk q3; ACT: q1, even-chunk
                    # q3 (+ the per-chunk exp) -- balances ~950ns/chunk each
                    on_dve = k % 2 == 0 or (k == 3 and m % 2 == 1)
                    if on_dve:
                        nc.vector.tensor_copy(out=o_sb[:, qk],
                                              in_=op_t[0:CH, qk])
                    else:
                        nc.scalar.activation(out=o_sb[:, qk],
                                             in_=op_t[0:CH, qk],
                                             func=ACT.Copy)
                    if k % 2 == 1:
                        # flush each completed half right away: finer DMA
                        # quanta interleave with gather bursts
                        hs = slice((k - 1) * NQ, (k + 1) * NQ)
                        nc.sync.dma_start(
                            out=out_d[m * CH:(m + 1) * CH, hs],
                            in_=o_sb[:, hs])

            # software-pipelined prologue (gathers prefetched PF deep)
            PF = 4
            for g0 in range(0, min(PF, NCH), GB):
                issue_gather(g0)
            issue_weights(0)

            for c in range(NCH):
                # gather prefetch (Pool only does gather gen now)
                if c % GB == 0 and c + PF < NCH:
                    issue_gather(c + PF)
                # previous chunk's carry + output FIRST: its ops head every
                # engine queue, so the serial carry->copy->carry path never
                # waits behind this iteration's prefetch work
                if c > 0:
                    finish_chunk(c - 1)
                if c + 1 < NCH:
                    issue_weights(c + 1)
                ttm = wt_tiles.pop(c)
                xg_t, xg_g = xg_tiles.pop(c)

                # main matmul (reversed rows; row 0 = chunk end, sans carry)
                op_t = outp.tile([128, D_], F32, tag="op")
                op_tiles[c] = op_t
                for h in range(NH):
                    sl = slice(h * NSPL, (h + 1) * NSPL)
                    nc.tensor.matmul(out=op_t[0:CH, sl], lhsT=ttm[:],
                                     rhs=xg_t[:, xg_g, sl],
                                     start=True, stop=True)


            finish_chunk(NCH - 1)

    nc.compile()
    names = dict(x=x_d.name, s16=s16_d.name, big=big_d.name, pm=pm_d.name,
                 out=out_d.name)
    return nc, names


def make_consts():
    ident = np.eye(128, dtype=np.float32)
    rev = np.eye(128, dtype=np.float32)[::-1].copy()
    jj = np.arange(128)
    # reversed triangular mask: out-row i' holds position (127 - i')
    mnegr = np.where(jj[:, None] > 127 - jj[None, :], NEG_BIG, 0.0).astype(
        np.float32)
    p16 = np.arange(16)
    le16 = (p16[:, None] <= p16[None, :]).astype(np.float16)
    gt16 = (p16[:, None] > p16[None, :]).astype(np.float16)
    rep16 = (p16[:, None] == (np.arange(128) % 16)[None, :]).astype(
        np.float16)
    big = np.concatenate([ident, rev, mnegr], axis=1)
    return dict(big=big, le16=le16, gt16=gt16, rep16=rep16)


_CACHE = {}


def _get_program():
    if "prog" not in _CACHE:
        _CACHE["prog"] = build_program()
    return _CACHE["prog"]


def per_core_inputs(names, hidden_b, bprob_b, mask_b, L_=L):
    import ml_dtypes

    NCH = L_ // 128
    NF = L_ // 16
    cs = make_consts()
    mf = mask_b.astype(np.float32)
    s16 = np.concatenate(
        [cs["le16"], cs["gt16"], cs["rep16"],
         np.ascontiguousarray(mf.reshape(NF, 16).T).astype(np.float16)],
        axis=1)
    pm = np.concatenate([np.ascontiguousarray(
        bprob_b[:, 1].reshape(NCH, 128)), mf.reshape(NCH, 128)], axis=1)
    return {
        names["x"]: np.ascontiguousarray(hidden_b).astype(ml_dtypes.bfloat16),
        names["s16"]: np.ascontiguousarray(s16),
        names["big"]: np.ascontiguousarray(cs["big"]),
        names["pm"]: np.ascontiguousarray(pm),
    }


def kernel(hidden_states, boundary_prob, boundary_mask):
    from concourse import bass_utils

    nc, names = _get_program()

    hidden_states = np.asarray(hidden_states, dtype=np.float32)
    boundary_prob = np.asarray(boundary_prob, dtype=np.float32)
    boundary_mask = np.asarray(boundary_mask)

    in_maps = [per_core_inputs(names, hidden_states[b], boundary_prob[b],
                               boundary_mask[b]) for b in range(B)]
    res = bass_utils.run_bass_kernel_spmd(nc, in_maps,
                                          core_ids=list(range(N_CORES)))
    out = np.stack([np.asarray(res.results[b][names["out"]]).astype(np.float32)
                    for b in range(B)], axis=0)
    # un-flip the per-chunk row reversal (device writes chunk rows reversed)
    out = out.reshape(B, L // 128, 128, D)[:, :, ::-1, :].reshape(B, L, D)
    return np.ascontiguousarray(out, dtype=np.float32)



# revision 4
# speedup vs baseline: 1375.6468x; 1375.6468x over previous
"""Trainium2 Bass kernel for nn_DeChunkLayer (H-Net dechunk: EMA over chunks +
broadcast back to token positions).

Formulation: instead of (argsort -> EMA over M -> gather back to L), run ONE
first-order linear recurrence over the L-length token axis:
    a_l = mask_l ? (1 - p_l) : 1
    b_l = mask_l ? p_l * x[pbi_l] : 0        (pbi = cumsum(mask) - 1)
    H_l = a_l * H_{l-1} + b_l
Then out[l] = H_l exactly (at the m-th boundary H becomes ema[m]; in between it
holds). No argsort/compaction and no output gather; the only data-dependent
movement is the row gather x[pbi_l], done with the HW-accelerated dma_gather.

Per chunk of 128 positions the recurrence is solved with matmuls:
    out[i] = sum_{j<=i} exp(S_i - S_j + ln s_j) * x[pbi_j]  +  exp(S_i) * H_prev
(S = within-chunk inclusive cumsum of log a; ln s folds the s_j scale and the
boundary mask into the exp bias). The chunk is laid out REVERSED on the output
partitions (row 0 = chunk end), which makes the inter-chunk state h_c EQUAL
the chunk's final output row 0: the carry rank-1 matmul for chunk m reads its
rhs directly from row 0 of chunk m-1's staged output tile in SBUF -- there is
no separate H-chain op at all.

Performance structure (sim: 320us baseline -> 117us):
 - bf16 data path (tolerance is 2e-2): x ships bf16 from the host, the gather
   moves 2KB rows, the Exp activation emits bf16 weights, and the output is
   written bf16 and upconverted on the host. fp8 x was tried and FAILS the
   tolerance (e3m4's 3.1% relerr lands on max-magnitude outputs).
 - The triangular -inf mask is preloaded into PSUM via a transpose-matmul of
   the (symmetric) mask matrix, so weight build is PE+ACT only.
 - All 64 carry rows (esr) are computed in ONE batched setup exp and bounced
   through DRAM into a single partition-0 row (matmul lhsT must sit at
   partition base 0).
 - Software pipelining: weights for chunk c+1 and the gather for chunk c+PF
   are issued before chunk c's dependent tail; chunk m's carry+copies+DMA are
   issued one iteration late so no engine queue head-of-line blocks the
   serial carry[q] -> copy[q] -> carry[q] path. Carries and PSUM->SBUF copies
   run at aligned quarter granularity; DVE and ACT split the quarters
   ~950ns/chunk each, with output DMA flushed per half.
 - GPSIMD (Pool) cannot touch PSUM; it only runs gather descriptor gen,
   batched GB=2 chunks per call (994ns fixed SWDGE overhead per call) and
   interleaved finely with the output DMAs on the shared DMA engines.
 - Setup: the gather-index pipeline (wrapped-16 cumsum -> int16 idx) runs
   first, in fp16 (values <= 2048 exact), from one packed input DMA, so the
   first gather fires ~6us in while the weights path streams in behind.

Sharded over batch: core b handles batch row b.
"""

import numpy as np

import concourse.bass as bass
import concourse.tile as tile
from concourse import bacc, mybir
from concourse.bass import IndirectOffsetOnAxis

F32 = mybir.dt.float32
BF16 = mybir.dt.bfloat16
FP16 = mybir.dt.float16
I16 = mybir.dt.int16
I32 = mybir.dt.int32
AX = mybir.AluOpType
ACT = mybir.ActivationFunctionType

# Problem constants (hardcoded per contract)
B, L, D, M = 8, 8192, 1024, 2048
EPS = 1e-4
N_CORES = 8
NEG_BIG = -1e30


def build_program(L_=L, D_=D, M_=M, reps=1, gather_mode="antgather"):
    """Build the per-core Bass program. Returns (nc, names dict)."""
    CH = 128                       # chunk length (= matmul K)
    NCH = L_ // CH                 # number of chunks
    NF = L_ // 16                  # wrapped-16 index columns
    assert NCH * CH == L_
    NSPL = min(512, D_)            # matmul free-dim split (psum bank = 512 f32)
    NH = D_ // NSPL

    from contextlib import ExitStack

    nc = bacc.Bacc(None, target_bir_lowering=False, debug=False)
    with tile.TileContext(nc) as tc, ExitStack() as ctx:
        dram = ctx.enter_context(tc.tile_pool(name="dram", bufs=1, space="DRAM"))
        x_d = dram.tile([M_, D_], BF16, kind="ExternalInput")
        # packed setup inputs: one DMA each (HWDGE gen is per-partition-count)
        s16_d = dram.tile([16, 160 + NF], FP16, kind="ExternalInput")
        big_d = dram.tile([128, 384], F32, kind="ExternalInput")
        pm_d = dram.tile([NCH, 2 * CH], F32, kind="ExternalInput")
        out_d = dram.tile([L_, D_], BF16, kind="ExternalOutput")

        setup = ctx.enter_context(tc.tile_pool(name="setup", bufs=1))
        bsp = ctx.enter_context(tc.tile_pool(name="bsp", bufs=2, space="PSUM"))
        xgp = ctx.enter_context(tc.tile_pool(name="xgp", bufs=3))
        ttp = ctx.enter_context(tc.tile_pool(name="ttp", bufs=3))
        osb = ctx.enter_context(tc.tile_pool(name="osb", bufs=10))

        # ---------------- setup ----------------
        # The gather-index pipeline (m16 -> c16 -> idx16) loads and computes
        # FIRST (in fp16: all values <= 2048 are exact) so the first
        # dma_gather starts while the rest of setup streams in behind it.
        s16 = setup.tile([16, 160 + NF], FP16)
        nc.sync.dma_start(out=s16[:], in_=s16_d[:])
        le16 = s16[:, 0:16]
        gt16 = s16[:, 16:32]
        rep16 = s16[:, 32:160]
        m16 = s16[:, 160:160 + NF]
        ones16 = setup.tile([16, 1], FP16)
        nc.vector.memset(ones16[:], 1.0)

        # FAST PATH: the first PF gathers need only the first 32 wrapped
        # columns; compute those in a separate small tile so the first
        # gather fires ~6us earlier (per-partition cumsum is column-local)
        FW = 32
        idx16a = setup.tile([128, FW], I16)
        idx16b = setup.tile([128, NF - FW], I16)
        with tc.tile_pool(name="bsps", bufs=1, space="PSUM") as bsps:
            cm = setup.tile([16, FW], FP16)
            nc.vector.tensor_tensor_scan(
                out=cm[:], data0=ones16[:].to_broadcast([16, FW]),
                data1=m16[:, 0:FW],
                initial=0.0, op0=AX.mult, op1=AX.add)
            mini_ps = bsps.tile([16, FW], F32, tag="bsm")
            nc.tensor.matmul(out=mini_ps[0:16, 0:FW], lhsT=le16,
                             rhs=cm[:], start=True, stop=False,
                             skip_group_check=True)
            nc.tensor.matmul(out=mini_ps[0:16, 1:FW], lhsT=gt16,
                             rhs=cm[0:16, 0:FW - 1],
                             start=False, stop=True, skip_group_check=True)
            pbi_m = setup.tile([16, FW], FP16)
            nc.vector.tensor_scalar_add(out=pbi_m[:],
                                        in0=mini_ps[0:16, 0:FW],
                                        scalar1=-1.0)
            minir_ps = bsps.tile([128, FW], F32, tag="bsmr")
            nc.tensor.matmul(out=minir_ps[0:128, 0:FW], lhsT=rep16,
                             rhs=pbi_m[:], start=True, stop=True)
            nc.vector.tensor_copy(out=idx16a[:],
                                  in_=minir_ps[0:128, 0:FW])

            # full-width path for the remaining columns
            c16 = setup.tile([16, NF], FP16)
            nc.vector.tensor_tensor_scan(
                out=c16[:], data0=ones16[:].to_broadcast([16, NF]),
                data1=m16,
                initial=0.0, op0=AX.mult, op1=AX.add)
            pbi16_ps = bsps.tile([16, NF], F32, tag="bs16")
            nc.tensor.matmul(out=pbi16_ps[0:16, FW:NF], lhsT=le16,
                             rhs=c16[:, FW:NF], start=True, stop=False,
                             skip_group_check=True)
            nc.tensor.matmul(out=pbi16_ps[0:16, FW:NF], lhsT=gt16,
                             rhs=c16[0:16, FW - 1:NF - 1],
                             start=False, stop=True, skip_group_check=True)
            pbi16 = setup.tile([16, NF], FP16)
            nc.vector.tensor_scalar_add(out=pbi16[0:16, FW:NF],
                                        in0=pbi16_ps[0:16, FW:NF],
                                        scalar1=-1.0)
            # replicate the 16 wrapped index rows to all 8 gpsimd core
            # slots with one fp16 matmul (values <= 2047: exact)
            idxrep_ps = bsps.tile([128, NF], F32, tag="bs16r")
            nc.tensor.matmul(out=idxrep_ps[0:128, FW:NF], lhsT=rep16,
                             rhs=pbi16[:, FW:NF], start=True, stop=True)
            nc.vector.tensor_copy(out=idx16b[:],
                                  in_=idxrep_ps[0:128, FW:NF])

        big = setup.tile([128, 384], F32)
        nc.sync.dma_start(out=big[:], in_=big_d[:])
        ident = big[:, 0:128]
        rev128 = big[:, 128:256]
        mnegr = big[:, 256:384]

        pm = setup.tile([NCH, 2 * CH], F32)
        nc.sync.dma_start(out=pm[:], in_=pm_d[:])
        praw = pm[:, 0:CH]
        mk = pm[:, CH:2 * CH]

        ones_r = setup.tile([NCH, CH], F32)
        nc.vector.memset(ones_r[:], 1.0)

        pc = setup.tile([NCH, CH], F32)
        nc.vector.tensor_scalar(out=pc[:], in0=praw, scalar1=EPS,
                                scalar2=1.0 - EPS, op0=AX.max, op1=AX.min)
        q = setup.tile([NCH, CH], F32)
        nc.vector.tensor_scalar(out=q[:], in0=pc[:], scalar1=-1.0,
                                scalar2=1.0, op0=AX.mult, op1=AX.add)
        lnq = setup.tile([NCH, CH], F32)
        nc.scalar.activation(out=lnq[:], in_=q[:], func=ACT.Ln)
        loga = setup.tile([NCH, CH], F32)
        nc.vector.tensor_tensor(out=loga[:], in0=lnq[:], in1=mk, op=AX.mult)

        # ln(s) with the mask folded in: ln(p) where mask else -1e30
        lnp = setup.tile([NCH, CH], F32)
        nc.scalar.activation(out=lnp[:], in_=pc[:], func=ACT.Ln)
        lnp_m = setup.tile([NCH, CH], F32)
        nc.vector.tensor_tensor(out=lnp_m[:], in0=lnp[:], in1=mk, op=AX.mult)
        mgate = setup.tile([NCH, CH], F32)
        nc.vector.tensor_scalar(out=mgate[:], in0=mk, scalar1=-NEG_BIG,
                                scalar2=NEG_BIG, op0=AX.mult, op1=AX.add)
        lns = setup.tile([NCH, CH], F32)
        nc.vector.tensor_tensor(out=lns[:], in0=lnp_m[:], in1=mgate[:],
                                op=AX.add)

        # within-chunk inclusive cumsum of log(a) (along free dim)
        s_i = setup.tile([NCH, CH], F32)
        nc.vector.tensor_tensor_scan(out=s_i[:], data0=ones_r[:], data1=loga[:],
                                     initial=0.0, op0=AX.mult, op1=AX.add)

        # indices in [CH, NCH] int32 layout (for the indirect_dma fallback)
        c_i = setup.tile([NCH, CH], F32)
        nc.vector.tensor_tensor_scan(out=c_i[:], data0=ones_r[:], data1=mk[:],
                                     initial=0.0, op0=AX.mult, op1=AX.add)
        cnt_colT = bsp.tile([128, 128], F32, tag="bs")
        nc.tensor.transpose(out=cnt_colT[0:1, 0:NCH], in_=c_i[:, CH - 1:CH],
                            identity=big[0:NCH, 0:NCH])
        cnt_row = setup.tile([1, NCH], F32)
        nc.vector.tensor_copy(out=cnt_row[:], in_=cnt_colT[0:1, 0:NCH])
        ones1 = setup.tile([1, 128], F32)
        nc.vector.memset(ones1[:], 1.0)
        cum_row = setup.tile([1, NCH], F32)
        nc.vector.tensor_tensor_scan(out=cum_row[:], data0=ones1[0:1, 0:NCH],
                                     data1=cnt_row[:], initial=0.0,
                                     op0=AX.mult, op1=AX.add)
        bases_row = setup.tile([1, NCH], F32)
        nc.vector.memset(bases_row[:], 0.0)
        nc.vector.tensor_copy(out=bases_row[0:1, 1:NCH],
                              in_=cum_row[0:1, 0:NCH - 1])
        bases_colT = bsp.tile([128, 128], F32, tag="bs")
        nc.tensor.transpose(out=bases_colT[0:NCH, 0:1], in_=bases_row[:],
                            identity=big[0:1, 0:1])
        bases_col = setup.tile([NCH, 1], F32)
        nc.vector.tensor_copy(out=bases_col[:], in_=bases_colT[0:NCH, 0:1])
        pbi_i = setup.tile([NCH, CH], F32)
        nc.vector.tensor_scalar(out=pbi_i[:], in0=c_i[:], scalar1=bases_col[:],
                                scalar2=-1.0, op0=AX.add, op1=AX.add)
        pbiT_ps = bsp.tile([128, 128], F32, tag="bs")
        nc.tensor.transpose(out=pbiT_ps[0:CH, 0:NCH], in_=pbi_i[:],
                            identity=big[0:NCH, 0:NCH])
        idxT = setup.tile([CH, NCH], I32)
        nc.vector.tensor_copy(out=idxT[:], in_=pbiT_ps[0:CH, 0:NCH])

        # transposed per-chunk columns: S, and bias = ln(s) - S
        ST_ps = bsp.tile([128, 128], F32, tag="bs")
        nc.tensor.transpose(out=ST_ps[0:CH, 0:NCH], in_=s_i[:],
                            identity=big[0:NCH, 0:NCH])
        ST = setup.tile([CH, NCH], F32)
        nc.vector.tensor_copy(out=ST[:], in_=ST_ps[0:CH, 0:NCH])
        lnsT_ps = bsp.tile([128, 128], F32, tag="bs")
        nc.tensor.transpose(out=lnsT_ps[0:CH, 0:NCH], in_=lns[:],
                            identity=big[0:NCH, 0:NCH])
        nbT = setup.tile([CH, NCH], F32)
        nc.vector.tensor_tensor(out=nbT[:], in0=lnsT_ps[0:CH, 0:NCH],
                                in1=ST[:], op=AX.subtract)

        # all carry rows at once: esr_all[c, i'] = exp(S^c_{127-i'})
        # (ST already holds s_i transposed; multiply by rev to flip free dim).
        # Matmul lhsT must sit at partition base 0, so bounce the [NCH, CH]
        # tile through DRAM and reload it as one [1, NCH*CH] partition-0 row.
        srev_ps = bsp.tile([128, 128], F32, tag="bs")
        nc.tensor.matmul(out=srev_ps[0:NCH, 0:CH], lhsT=ST[:],
                         rhs=rev128, is_transpose=True,
                         start=True, stop=True)
        esr_all = setup.tile([NCH, CH], BF16)
        nc.scalar.activation(out=esr_all[:], in_=srev_ps[0:NCH, 0:CH],
                             func=ACT.Exp)
        esr_d = dram.tile([NCH, CH], BF16)
        nc.sync.dma_start(out=esr_d[:], in_=esr_all[:])
        esr_row = setup.tile([1, NCH * CH], BF16)
        nc.sync.dma_start(out=esr_row[:],
                          in_=esr_d[:].rearrange("a b -> (a b)"))

        # ---------------- main loop ----------------
        # created after the setup's scoped psum pool is released: 2 (bs)
        # + 3*2 (outp) = 8 banks
        outp = ctx.enter_context(tc.tile_pool(name="outp", bufs=3,
                                              space="PSUM"))
        GB = 2                      # chunks per batched gather call

        for _rep in range(reps):
            xg_tiles = {}
            wt_tiles = {}

            def issue_gather(c0):
                """One SWDGE call gathers GB chunks (994ns fixed overhead per
                call); group g of the out tile = chunk c0+g."""
                if gather_mode == "antgather":
                    xgb = xgp.tile([CH, GB, D_], BF16, tag="xg")
                    if 8 * (c0 + GB) <= FW:
                        idxs = idx16a[:, 8 * c0:8 * (c0 + GB)]
                    else:
                        idxs = idx16b[:, 8 * c0 - FW:8 * (c0 + GB) - FW]
                    nc.gpsimd.dma_gather(
                        out_ap=xgb[:],
                        in_ap=x_d[:],
                        idxs_ap=idxs,
                        num_idxs=CH * GB, num_idxs_reg=CH * GB,
                        elem_size=D_)
                    for g in range(GB):
                        xg_tiles[c0 + g] = (xgb, g)
                elif gather_mode == "indirect":
                    for g in range(GB):
                        xgt = xgp.tile([CH, 1, D_], BF16, tag="xgs")
                        nc.gpsimd.indirect_dma_start(
                            out=xgt[:, 0, :], out_offset=None, in_=x_d[:],
                            in_offset=IndirectOffsetOnAxis(
                                ap=idxT[:, c0 + g:c0 + g + 1], axis=0))
                        xg_tiles[c0 + g] = (xgt, 0)
                else:
                    raise ValueError(gather_mode)

            def issue_weights(c):
                """PSUM <- mnegr (symmetric triangular -inf mask), then
                accumulate Sbc[j, i'] = S_{127-i'} (PE transpose of the
                free-broadcast S column against the anti-diagonal perm);
                then weights ttm[j, i'] = exp(S_{127-i'} - S_j + ln s_j)."""
                sbc = bsp.tile([128, 128], F32, tag="bs")
                nc.tensor.matmul(out=sbc[0:CH, 0:CH], lhsT=mnegr,
                                 rhs=ident, is_transpose=True,
                                 start=True, stop=False, skip_group_check=True)
                nc.tensor.matmul(out=sbc[0:CH, 0:CH],
                                 lhsT=ST[:, c:c + 1].to_broadcast([CH, CH]),
                                 rhs=rev128, is_transpose=True,
                                 start=False, stop=True, skip_group_check=True)
                ttm = ttp.tile([CH, CH], BF16, tag="ttm")
                nc.scalar.activation(out=ttm[:], in_=sbc[0:CH, 0:CH],
                                     func=ACT.Exp, bias=nbT[:, c:c + 1])
                wt_tiles[c] = ttm

            op_tiles = {}
            o_tiles = {}

            NQ = D_ // 4

            def finish_chunk(m):
                """Carry accumulation + output for chunk m, issued one
                iteration late. The carry rhs is row 0 of the PREVIOUS chunk's
                staged output: with reversed rows, out row 0 = position 127 =
                H at chunk end including its own carry = h_{m-1}, already in
                SBUF bf16 -- no separate H-chain op needed. Carry matmuls and
                PSUM->SBUF copies run at aligned QUARTER granularity: each
                quarter is an independent serial sub-path
                (carry[qk] -> copy[qk] -> next carry[qk]) whose copy engine
                (DVE for even quarters, ACT for odd) never gates another
                quarter's path."""
                op_t = op_tiles.pop(m)
                o_sb = osb.tile([CH, D_], BF16, tag="osb")
                o_tiles[m] = o_sb
                prev = o_tiles.pop(m - 1, None)
                for k in range(4):
                    qk = slice(k * NQ, (k + 1) * NQ)
                    if m > 0:
                        nc.tensor.matmul(out=op_t[0:CH, qk],
                                         lhsT=esr_row[0:1,
                                                      m * CH:(m + 1) * CH],
                                         rhs=prev[0:1, qk],
                                         start=False, stop=True,
                                         skip_group_check=True)
                    # DVE: q0, q2 and odd-chunk q3; ACT: q1, even-chunk
                    # q3 (+ the per-chunk exp) -- balances ~950ns/chunk each
                    on_dve = k % 2 == 0 or (k == 3 and m % 2 == 1)
                    if on_dve:
                        nc.vector.tensor_copy(out=o_sb[:, qk],
                                              in_=op_t[0:CH, qk])
                    else:
                        nc.scalar.activation(out=o_sb[:, qk],
                                             in_=op_t[0:CH, qk],
                                             func=ACT.Copy)
                    if k % 2 == 1:
                        # flush each completed half right away: finer DMA
                        # quanta interleave with gather bursts
                        hs = slice((k - 1) * NQ, (k + 1) * NQ)
                        nc.sync.dma_start(
                            out=out_d[m * CH:(m + 1) * CH, hs],
                            in_=o_sb[:, hs])

            # software-pipelined prologue (gathers prefetched PF deep)
            PF = 4
            for g0 in range(0, min(PF, NCH), GB):
                issue_gather(g0)
            issue_weights(0)

            for c in range(NCH):
                # gather prefetch (Pool only does gather gen now)
                if c % GB == 0 and c + PF < NCH:
                    issue_gather(c + PF)
                # previous chunk's carry + output FIRST: its ops head every
                # engine queue, so the serial carry->copy->carry path never
                # waits behind this iteration's prefetch work
                if c > 0:
                    finish_chunk(c - 1)
                if c + 1 < NCH:
                    issue_weights(c + 1)
                ttm = wt_tiles.pop(c)
                xg_t, xg_g = xg_tiles.pop(c)

                # main matmul (reversed rows; row 0 = chunk end, sans carry)
                op_t = outp.tile([128, D_], F32, tag="op")
                op_tiles[c] = op_t
                for h in range(NH):
                    sl = slice(h * NSPL, (h + 1) * NSPL)
                    nc.tensor.matmul(out=op_t[0:CH, sl], lhsT=ttm[:],
                                     rhs=xg_t[:, xg_g, sl],
                                     start=True, stop=True)


            finish_chunk(NCH - 1)

    nc.compile()
    names = dict(x=x_d.name, s16=s16_d.name, big=big_d.name, pm=pm_d.name,
                 out=out_d.name)
    return nc, names


def make_consts():
    ident = np.eye(128, dtype=np.float32)
    rev = np.eye(128, dtype=np.float32)[::-1].copy()
    jj = np.arange(128)
    # reversed triangular mask: out-row i' holds position (127 - i')
    mnegr = np.where(jj[:, None] > 127 - jj[None, :], NEG_BIG, 0.0).astype(
        np.float32)
    p16 = np.arange(16)
    le16 = (p16[:, None] <= p16[None, :]).astype(np.float16)
    gt16 = (p16[:, None] > p16[None, :]).astype(np.float16)
    rep16 = (p16[:, None] == (np.arange(128) % 16)[None, :]).astype(
        np.float16)
    big = np.concatenate([ident, rev, mnegr], axis=1)
    return dict(big=big, le16=le16, gt16=gt16, rep16=rep16)


_CACHE = {}


def _get_program():
    if "prog" not in _CACHE:
        _CACHE["prog"] = build_program()
    return _CACHE["prog"]


def per_core_inputs(names, hidden_b, bprob_b, mask_b, L_=L):
    import ml_dtypes

    NCH = L_ // 128
    NF = L_ // 16
    cs = make_consts()
    mf = mask_b.astype(np.float32)
    s16 = np.concatenate(
        [cs["le16"], cs["gt16"], cs["rep16"],
         np.ascontiguousarray(mf.reshape(NF, 16).T).astype(np.float16)],
        axis=1)
    pm = np.concatenate([np.ascontiguousarray(
        bprob_b[:, 1].reshape(NCH, 128)), mf.reshape(NCH, 128)], axis=1)
    return {
        names["x"]: np.ascontiguousarray(hidden_b).astype(ml_dtypes.bfloat16),
        names["s16"]: np.ascontiguousarray(s16),
        names["big"]: np.ascontiguousarray(cs["big"]),
        names["pm"]: np.ascontiguousarray(pm),
    }


def kernel(hidden_states, boundary_prob, boundary_mask):
    from concourse import bass_utils

    nc, names = _get_program()

    hidden_states = np.asarray(hidden_states, dtype=np.float32)
    boundary_prob = np.asarray(boundary_prob, dtype=np.float32)
    boundary_mask = np.asarray(boundary_mask)

    in_maps = [per_core_inputs(names, hidden_states[b], boundary_prob[b],
                               boundary_mask[b]) for b in range(B)]
    res = bass_utils.run_bass_kernel_spmd(nc, in_maps,
                                          core_ids=list(range(N_CORES)))
    out = np.stack([np.asarray(res.results[b][names["out"]]).astype(np.float32)
                    for b in range(B)], axis=0)
    # un-flip the per-chunk row reversal (device writes chunk rows reversed)
    out = out.reshape(B, L // 128, 128, D)[:, :, ::-1, :].reshape(B, L, D)
    return np.ascontiguousarray(out, dtype=np.float32)



# revision 5
# speedup vs baseline: 1447.4661x; 1.0522x over previous
"""Trainium2 Bass kernel for nn_DeChunkLayer (H-Net dechunk: EMA over chunks +
broadcast back to token positions).

Formulation: instead of (argsort -> EMA over M -> gather back to L), run ONE
first-order linear recurrence over the L-length token axis:
    a_l = mask_l ? (1 - p_l) : 1
    b_l = mask_l ? p_l * x[pbi_l] : 0        (pbi = cumsum(mask) - 1)
    H_l = a_l * H_{l-1} + b_l
Then out[l] = H_l exactly (at the m-th boundary H becomes ema[m]; in between it
holds). No argsort/compaction and no output gather; the only data-dependent
movement is the row gather x[pbi_l], done with the HW-accelerated dma_gather.

Per chunk of 128 positions the recurrence is solved with matmuls:
    out[i] = sum_{j<=i} exp(S_i - S_j + ln s_j) * x[pbi_j]  +  exp(S_i) * H_prev
(S = within-chunk inclusive cumsum of log a; ln s folds the s_j scale and the
boundary mask into the exp bias). The chunk is laid out REVERSED on the output
partitions (row 0 = chunk end), which makes the inter-chunk state h_c EQUAL
the chunk's final output row 0: the carry rank-1 matmul for chunk m reads its
rhs directly from row 0 of chunk m-1's staged output tile in SBUF -- there is
no separate H-chain op at all.

Performance structure (sim: 320us baseline -> 117us):
 - bf16 data path (tolerance is 2e-2): x ships bf16 from the host, the gather
   moves 2KB rows, the Exp activation emits bf16 weights, and the output is
   written bf16 and upconverted on the host. fp8 x was tried and FAILS the
   tolerance (e3m4's 3.1% relerr lands on max-magnitude outputs).
 - The triangular -inf mask is preloaded into PSUM via a transpose-matmul of
   the (symmetric) mask matrix, so weight build is PE+ACT only.
 - All 64 carry rows (esr) are computed in ONE batched setup exp and bounced
   through DRAM into a single partition-0 row (matmul lhsT must sit at
   partition base 0).
 - Software pipelining: weights for chunk c+1 and the gather for chunk c+PF
   are issued before chunk c's dependent tail; chunk m's carry+copies+DMA are
   issued one iteration late so no engine queue head-of-line blocks the
   serial carry[q] -> copy[q] -> carry[q] path. Carries and PSUM->SBUF copies
   run at aligned quarter granularity; DVE and ACT split the quarters
   ~950ns/chunk each, with output DMA flushed per half.
 - GPSIMD (Pool) cannot touch PSUM; it only runs gather descriptor gen,
   batched GB=2 chunks per call (994ns fixed SWDGE overhead per call) and
   interleaved finely with the output DMAs on the shared DMA engines.
 - Setup: the gather-index pipeline (wrapped-16 cumsum -> int16 idx) runs
   first, in fp16 (values <= 2048 exact), from one packed input DMA, so the
   first gather fires ~6us in while the weights path streams in behind.

Sharded over batch: core b handles batch row b.
"""

import numpy as np

import concourse.bass as bass
import concourse.tile as tile
from concourse import bacc, mybir
from concourse.bass import IndirectOffsetOnAxis

F32 = mybir.dt.float32
BF16 = mybir.dt.bfloat16
FP16 = mybir.dt.float16
I16 = mybir.dt.int16
I32 = mybir.dt.int32
AX = mybir.AluOpType
ACT = mybir.ActivationFunctionType

# Problem constants (hardcoded per contract)
B, L, D, M = 8, 8192, 1024, 2048
EPS = 1e-4
N_CORES = 8
NEG_BIG = -1e30


def build_program(L_=L, D_=D, M_=M, reps=1, gather_mode="antgather"):
    """Build the per-core Bass program. Returns (nc, names dict)."""
    CH = 128                       # chunk length (= matmul K)
    NCH = L_ // CH                 # number of chunks
    NF = L_ // 16                  # wrapped-16 index columns
    assert NCH * CH == L_
    NSPL = min(512, D_)            # matmul free-dim split (psum bank = 512 f32)
    NH = D_ // NSPL

    from contextlib import ExitStack

    nc = bacc.Bacc(None, target_bir_lowering=False, debug=False)
    with tile.TileContext(nc) as tc, ExitStack() as ctx:
        dram = ctx.enter_context(tc.tile_pool(name="dram", bufs=1, space="DRAM"))
        x_d = dram.tile([M_, D_], BF16, kind="ExternalInput")
        # packed setup inputs: one DMA each (HWDGE gen is per-partition-count)
        s16_d = dram.tile([16, 160 + NF], FP16, kind="ExternalInput")
        big_d = dram.tile([128, 384], F32, kind="ExternalInput")
        pm_d = dram.tile([NCH, 2 * CH], F32, kind="ExternalInput")
        out_d = dram.tile([L_, D_], BF16, kind="ExternalOutput")

        setup = ctx.enter_context(tc.tile_pool(name="setup", bufs=1))
        bsp = ctx.enter_context(tc.tile_pool(name="bsp", bufs=2, space="PSUM"))
        xgp = ctx.enter_context(tc.tile_pool(name="xgp", bufs=3))
        ttp = ctx.enter_context(tc.tile_pool(name="ttp", bufs=3))
        osb = ctx.enter_context(tc.tile_pool(name="osb", bufs=10))

        # ---------------- setup ----------------
        # The gather-index pipeline (m16 -> c16 -> idx16) loads and computes
        # FIRST (in fp16: all values <= 2048 are exact) so the first
        # dma_gather starts while the rest of setup streams in behind it.
        s16 = setup.tile([16, 160 + NF], FP16)
        nc.sync.dma_start(out=s16[:], in_=s16_d[:])
        le16 = s16[:, 0:16]
        gt16 = s16[:, 16:32]
        rep16 = s16[:, 32:160]
        m16 = s16[:, 160:160 + NF]
        ones16 = setup.tile([16, 1], FP16)
        nc.vector.memset(ones16[:], 1.0)

        # FAST PATH: the first PF gathers need only the first 32 wrapped
        # columns; compute those in a separate small tile so the first
        # gather fires ~6us earlier (per-partition cumsum is column-local)
        FW = 32
        idx16a = setup.tile([128, FW], I16)
        idx16b = setup.tile([128, NF - FW], I16)
        with tc.tile_pool(name="bsps", bufs=1, space="PSUM") as bsps:
            cm = setup.tile([16, FW], FP16)
            nc.vector.tensor_tensor_scan(
                out=cm[:], data0=ones16[:].to_broadcast([16, FW]),
                data1=m16[:, 0:FW],
                initial=0.0, op0=AX.mult, op1=AX.add)
            mini_ps = bsps.tile([16, FW], F32, tag="bsm")
            nc.tensor.matmul(out=mini_ps[0:16, 0:FW], lhsT=le16,
                             rhs=cm[:], start=True, stop=False,
                             skip_group_check=True)
            nc.tensor.matmul(out=mini_ps[0:16, 1:FW], lhsT=gt16,
                             rhs=cm[0:16, 0:FW - 1],
                             start=False, stop=True, skip_group_check=True)
            pbi_m = setup.tile([16, FW], FP16)
            nc.vector.tensor_scalar_add(out=pbi_m[:],
                                        in0=mini_ps[0:16, 0:FW],
                                        scalar1=-1.0)
            minir_ps = bsps.tile([128, FW], F32, tag="bsmr")
            nc.tensor.matmul(out=minir_ps[0:128, 0:FW], lhsT=rep16,
                             rhs=pbi_m[:], start=True, stop=True)
            nc.vector.tensor_copy(out=idx16a[:],
                                  in_=minir_ps[0:128, 0:FW])

            # full-width path for the remaining columns
            c16 = setup.tile([16, NF], FP16)
            nc.vector.tensor_tensor_scan(
                out=c16[:], data0=ones16[:].to_broadcast([16, NF]),
                data1=m16,
                initial=0.0, op0=AX.mult, op1=AX.add)
            pbi16_ps = bsps.tile([16, NF], F32, tag="bs16")
            nc.tensor.matmul(out=pbi16_ps[0:16, FW:NF], lhsT=le16,
                             rhs=c16[:, FW:NF], start=True, stop=False,
                             skip_group_check=True)
            nc.tensor.matmul(out=pbi16_ps[0:16, FW:NF], lhsT=gt16,
                             rhs=c16[0:16, FW - 1:NF - 1],
                             start=False, stop=True, skip_group_check=True)
            pbi16 = setup.tile([16, NF], FP16)
            nc.vector.tensor_scalar_add(out=pbi16[0:16, FW:NF],
                                        in0=pbi16_ps[0:16, FW:NF],
                                        scalar1=-1.0)
            # replicate the 16 wrapped index rows to all 8 gpsimd core
            # slots with one fp16 matmul (values <= 2047: exact)
            idxrep_ps = bsps.tile([128, NF], F32, tag="bs16r")
            nc.tensor.matmul(out=idxrep_ps[0:128, FW:NF], lhsT=rep16,
                             rhs=pbi16[:, FW:NF], start=True, stop=True)
            nc.vector.tensor_copy(out=idx16b[:],
                                  in_=idxrep_ps[0:128, FW:NF])

        big = setup.tile([128, 384], F32)
        nc.sync.dma_start(out=big[:], in_=big_d[:])
        ident = big[:, 0:128]
        rev128 = big[:, 128:256]
        mnegr = big[:, 256:384]

        pm = setup.tile([NCH, 2 * CH], F32)
        nc.sync.dma_start(out=pm[:], in_=pm_d[:])
        praw = pm[:, 0:CH]
        mk = pm[:, CH:2 * CH]

        ones_r = setup.tile([NCH, CH], F32)
        nc.vector.memset(ones_r[:], 1.0)

        pc = setup.tile([NCH, CH], F32)
        nc.vector.tensor_scalar(out=pc[:], in0=praw, scalar1=EPS,
                                scalar2=1.0 - EPS, op0=AX.max, op1=AX.min)
        q = setup.tile([NCH, CH], F32)
        nc.vector.tensor_scalar(out=q[:], in0=pc[:], scalar1=-1.0,
                                scalar2=1.0, op0=AX.mult, op1=AX.add)
        lnq = setup.tile([NCH, CH], F32)
        nc.scalar.activation(out=lnq[:], in_=q[:], func=ACT.Ln)
        loga = setup.tile([NCH, CH], F32)
        nc.vector.tensor_tensor(out=loga[:], in0=lnq[:], in1=mk, op=AX.mult)

        # ln(s) with the mask folded in: ln(p) where mask else -1e30
        lnp = setup.tile([NCH, CH], F32)
        nc.scalar.activation(out=lnp[:], in_=pc[:], func=ACT.Ln)
        lnp_m = setup.tile([NCH, CH], F32)
        nc.vector.tensor_tensor(out=lnp_m[:], in0=lnp[:], in1=mk, op=AX.mult)
        mgate = setup.tile([NCH, CH], F32)
        nc.vector.tensor_scalar(out=mgate[:], in0=mk, scalar1=-NEG_BIG,
                                scalar2=NEG_BIG, op0=AX.mult, op1=AX.add)
        lns = setup.tile([NCH, CH], F32)
        nc.vector.tensor_tensor(out=lns[:], in0=lnp_m[:], in1=mgate[:],
                                op=AX.add)

        # within-chunk inclusive cumsum of log(a) (along free dim)
        s_i = setup.tile([NCH, CH], F32)
        nc.vector.tensor_tensor_scan(out=s_i[:], data0=ones_r[:], data1=loga[:],
                                     initial=0.0, op0=AX.mult, op1=AX.add)

        # indices in [CH, NCH] int32 layout (for the indirect_dma fallback)
        c_i = setup.tile([NCH, CH], F32)
        nc.vector.tensor_tensor_scan(out=c_i[:], data0=ones_r[:], data1=mk[:],
                                     initial=0.0, op0=AX.mult, op1=AX.add)
        cnt_colT = bsp.tile([128, 128], F32, tag="bs")
        nc.tensor.transpose(out=cnt_colT[0:1, 0:NCH], in_=c_i[:, CH - 1:CH],
                            identity=big[0:NCH, 0:NCH])
        cnt_row = setup.tile([1, NCH], F32)
        nc.vector.tensor_copy(out=cnt_row[:], in_=cnt_colT[0:1, 0:NCH])
        ones1 = setup.tile([1, 128], F32)
        nc.vector.memset(ones1[:], 1.0)
        cum_row = setup.tile([1, NCH], F32)
        nc.vector.tensor_tensor_scan(out=cum_row[:], data0=ones1[0:1, 0:NCH],
                                     data1=cnt_row[:], initial=0.0,
                                     op0=AX.mult, op1=AX.add)
        bases_row = setup.tile([1, NCH], F32)
        nc.vector.memset(bases_row[:], 0.0)
        nc.vector.tensor_copy(out=bases_row[0:1, 1:NCH],
                              in_=cum_row[0:1, 0:NCH - 1])
        bases_colT = bsp.tile([128, 128], F32, tag="bs")
        nc.tensor.transpose(out=bases_colT[0:NCH, 0:1], in_=bases_row[:],
                            identity=big[0:1, 0:1])
        bases_col = setup.tile([NCH, 1], F32)
        nc.vector.tensor_copy(out=bases_col[:], in_=bases_colT[0:NCH, 0:1])
        pbi_i = setup.tile([NCH, CH], F32)
        nc.vector.tensor_scalar(out=pbi_i[:], in0=c_i[:], scalar1=bases_col[:],
                                scalar2=-1.0, op0=AX.add, op1=AX.add)
        pbiT_ps = bsp.tile([128, 128], F32, tag="bs")
        nc.tensor.transpose(out=pbiT_ps[0:CH, 0:NCH], in_=pbi_i[:],
                            identity=big[0:NCH, 0:NCH])
        idxT = setup.tile([CH, NCH], I32)
        nc.vector.tensor_copy(out=idxT[:], in_=pbiT_ps[0:CH, 0:NCH])

        # transposed per-chunk columns: S, and bias = ln(s) - S
        ST_ps = bsp.tile([128, 128], F32, tag="bs")
        nc.tensor.transpose(out=ST_ps[0:CH, 0:NCH], in_=s_i[:],
                            identity=big[0:NCH, 0:NCH])
        ST = setup.tile([CH, NCH], F32)
        nc.vector.tensor_copy(out=ST[:], in_=ST_ps[0:CH, 0:NCH])
        lnsT_ps = bsp.tile([128, 128], F32, tag="bs")
        nc.tensor.transpose(out=lnsT_ps[0:CH, 0:NCH], in_=lns[:],
                            identity=big[0:NCH, 0:NCH])
        nbT = setup.tile([CH, NCH], F32)
        nc.vector.tensor_tensor(out=nbT[:], in0=lnsT_ps[0:CH, 0:NCH],
                                in1=ST[:], op=AX.subtract)

        # all carry rows at once: esr_all[c, i'] = exp(S^c_{127-i'})
        # (ST already holds s_i transposed; multiply by rev to flip free dim).
        # Matmul lhsT must sit at partition base 0, so bounce the [NCH, CH]
        # tile through DRAM and reload it as one [1, NCH*CH] partition-0 row.
        srev_ps = bsp.tile([128, 128], F32, tag="bs")
        nc.tensor.matmul(out=srev_ps[0:NCH, 0:CH], lhsT=ST[:],
                         rhs=rev128, is_transpose=True,
                         start=True, stop=True)
        esr_all = setup.tile([NCH, CH], BF16)
        nc.scalar.activation(out=esr_all[:], in_=srev_ps[0:NCH, 0:CH],
                             func=ACT.Exp)
        esr_d = dram.tile([NCH, CH], BF16)
        nc.sync.dma_start(out=esr_d[:], in_=esr_all[:])
        esr_row = setup.tile([1, NCH * CH], BF16)
        nc.sync.dma_start(out=esr_row[:],
                          in_=esr_d[:].rearrange("a b -> (a b)"))

        # ---------------- main loop ----------------
        # created after the setup's scoped psum pool is released: 2 (bs)
        # + 3*2 (outp) = 8 banks
        outa = ctx.enter_context(tc.tile_pool(name="outa", bufs=3,
                                              space="PSUM"))
        outb = ctx.enter_context(tc.tile_pool(name="outb", bufs=3,
                                              space="PSUM"))
        GB = 2                      # chunks per batched gather call

        for _rep in range(reps):
            xg_tiles = {}
            wt_tiles = {}

            def issue_gather(c0):
                """One SWDGE call gathers GB chunks (994ns fixed overhead per
                call); group g of the out tile = chunk c0+g."""
                if gather_mode == "antgather":
                    xgb = xgp.tile([CH, GB, D_], BF16, tag="xg")
                    if 8 * (c0 + GB) <= FW:
                        idxs = idx16a[:, 8 * c0:8 * (c0 + GB)]
                    else:
                        idxs = idx16b[:, 8 * c0 - FW:8 * (c0 + GB) - FW]
                    nc.gpsimd.dma_gather(
                        out_ap=xgb[:],
                        in_ap=x_d[:],
                        idxs_ap=idxs,
                        num_idxs=CH * GB, num_idxs_reg=CH * GB,
                        elem_size=D_)
                    for g in range(GB):
                        xg_tiles[c0 + g] = (xgb, g)
                elif gather_mode == "indirect":
                    for g in range(GB):
                        xgt = xgp.tile([CH, 1, D_], BF16, tag="xgs")
                        nc.gpsimd.indirect_dma_start(
                            out=xgt[:, 0, :], out_offset=None, in_=x_d[:],
                            in_offset=IndirectOffsetOnAxis(
                                ap=idxT[:, c0 + g:c0 + g + 1], axis=0))
                        xg_tiles[c0 + g] = (xgt, 0)
                else:
                    raise ValueError(gather_mode)

            def issue_weights(c):
                """PSUM <- mnegr (symmetric triangular -inf mask), then
                accumulate Sbc[j, i'] = S_{127-i'} (PE transpose of the
                free-broadcast S column against the anti-diagonal perm);
                then weights ttm[j, i'] = exp(S_{127-i'} - S_j + ln s_j)."""
                sbc = bsp.tile([128, 128], F32, tag="bs")
                nc.tensor.matmul(out=sbc[0:CH, 0:CH], lhsT=mnegr,
                                 rhs=ident, is_transpose=True,
                                 start=True, stop=False, skip_group_check=True)
                nc.tensor.matmul(out=sbc[0:CH, 0:CH],
                                 lhsT=ST[:, c:c + 1].to_broadcast([CH, CH]),
                                 rhs=rev128, is_transpose=True,
                                 start=False, stop=True, skip_group_check=True)
                ttm = ttp.tile([CH, CH], BF16, tag="ttm")
                nc.scalar.activation(out=ttm[:], in_=sbc[0:CH, 0:CH],
                                     func=ACT.Exp, bias=nbT[:, c:c + 1])
                wt_tiles[c] = ttm

            op_tiles = {}
            o_tiles = {}

            NQ = D_ // 4

            def finish_chunk(m):
                """Carry accumulation + output for chunk m, issued one
                iteration late. The carry rhs is row 0 of the PREVIOUS chunk's
                staged output: with reversed rows, out row 0 = position 127 =
                H at chunk end including its own carry = h_{m-1}, already in
                SBUF bf16 -- no separate H-chain op needed. Carry matmuls and
                PSUM->SBUF copies run at aligned QUARTER granularity: each
                quarter is an independent serial sub-path
                (carry[qk] -> copy[qk] -> next carry[qk]) whose copy engine
                (DVE for even quarters, ACT for odd) never gates another
                quarter's path."""
                opa, opb = op_tiles.pop(m)
                o_sb = osb.tile([CH, D_], BF16, tag="osb")
                o_tiles[m] = o_sb
                prev = o_tiles.pop(m - 1, None)
                # q0/q1 (outa) on DVE, q2/q3 (outb) on ACT: the two copy
                # lanes never couple through a shared psum tile's WAR, so an
                # ACT lag (it also runs the weight exps) can't stall the
                # DVE-side carry chain or the next chunk's main matmul.
                for k in range(4):
                    qk = slice(k * NQ, (k + 1) * NQ)
                    half = opa if k < 2 else opb
                    hq = slice((k % 2) * NQ, (k % 2 + 1) * NQ)
                    if m > 0:
                        nc.tensor.matmul(out=half[0:CH, hq],
                                         lhsT=esr_row[0:1,
                                                      m * CH:(m + 1) * CH],
                                         rhs=prev[0:1, qk],
                                         start=False, stop=True,
                                         skip_group_check=True)
                    if k < 2:
                        nc.vector.tensor_copy(out=o_sb[:, qk],
                                              in_=half[0:CH, hq])
                    else:
                        nc.scalar.activation(out=o_sb[:, qk],
                                             in_=half[0:CH, hq],
                                             func=ACT.Copy)
                    if k % 2 == 1:
                        hs = slice((k - 1) * NQ, (k + 1) * NQ)
                        nc.sync.dma_start(
                            out=out_d[m * CH:(m + 1) * CH, hs],
                            in_=o_sb[:, hs])

            # software-pipelined prologue (gathers prefetched PF deep)
            PF = 4
            for g0 in range(0, min(PF, NCH), GB):
                issue_gather(g0)
            issue_weights(0)

            for c in range(NCH):
                # gather prefetch (Pool only does gather gen now)
                if c % GB == 0 and c + PF < NCH:
                    issue_gather(c + PF)
                # previous chunk's carry + output FIRST: its ops head every
                # engine queue, so the serial carry->copy->carry path never
                # waits behind this iteration's prefetch work
                if c > 0:
                    finish_chunk(c - 1)
                if c + 1 < NCH:
                    issue_weights(c + 1)
                ttm = wt_tiles.pop(c)
                xg_t, xg_g = xg_tiles.pop(c)

                # main matmul (reversed rows; row 0 = chunk end, sans carry)
                opa = outa.tile([128, NSPL], F32, tag="opa")
                opb = outb.tile([128, NSPL], F32, tag="opb")
                op_tiles[c] = (opa, opb)
                for h, half in enumerate((opa, opb)):
                    sl = slice(h * NSPL, (h + 1) * NSPL)
                    nc.tensor.matmul(out=half[0:CH, 0:NSPL], lhsT=ttm[:],
                                     rhs=xg_t[:, xg_g, sl],
                                     start=True, stop=True)


            finish_chunk(NCH - 1)

    nc.compile()
    names = dict(x=x_d.name, s16=s16_d.name, big=big_d.name, pm=pm_d.name,
                 out=out_d.name)
    return nc, names


def make_consts():
    ident = np.eye(128, dtype=np.float32)
    rev = np.eye(128, dtype=np.float32)[::-1].copy()
    jj = np.arange(128)
    # reversed triangular mask: out-row i' holds position (127 - i')
    mnegr = np.where(jj[:, None] > 127 - jj[None, :], NEG_BIG, 0.0).astype(
        np.float32)
    p16 = np.arange(16)
    le16 = (p16[:, None] <= p16[None, :]).astype(np.float16)
    gt16 = (p16[:, None] > p16[None, :]).astype(np.float16)
    rep16 = (p16[:, None] == (np.arange(128) % 16)[None, :]).astype(
        np.float16)
    big = np.concatenate([ident, rev, mnegr], axis=1)
    return dict(big=big, le16=le16, gt16=gt16, rep16=rep16)


_CACHE = {}


def _get_program():
    if "prog" not in _CACHE:
        _CACHE["prog"] = build_program()
    return _CACHE["prog"]


def per_core_inputs(names, hidden_b, bprob_b, mask_b, L_=L):
    import ml_dtypes

    NCH = L_ // 128
    NF = L_ // 16
    cs = make_consts()
    mf = mask_b.astype(np.float32)
    s16 = np.concatenate(
        [cs["le16"], cs["gt16"], cs["rep16"],
         np.ascontiguousarray(mf.reshape(NF, 16).T).astype(np.float16)],
        axis=1)
    pm = np.concatenate([np.ascontiguousarray(
        bprob_b[:, 1].reshape(NCH, 128)), mf.reshape(NCH, 128)], axis=1)
    return {
        names["x"]: np.ascontiguousarray(hidden_b).astype(ml_dtypes.bfloat16),
        names["s16"]: np.ascontiguousarray(s16),
        names["big"]: np.ascontiguousarray(cs["big"]),
        names["pm"]: np.ascontiguousarray(pm),
    }


def kernel(hidden_states, boundary_prob, boundary_mask):
    from concourse import bass_utils

    nc, names = _get_program()

    hidden_states = np.asarray(hidden_states, dtype=np.float32)
    boundary_prob = np.asarray(boundary_prob, dtype=np.float32)
    boundary_mask = np.asarray(boundary_mask)

    in_maps = [per_core_inputs(names, hidden_states[b], boundary_prob[b],
                               boundary_mask[b]) for b in range(B)]
    res = bass_utils.run_bass_kernel_spmd(nc, in_maps,
                                          core_ids=list(range(N_CORES)))
    out = np.stack([np.asarray(res.results[b][names["out"]]).astype(np.float32)
                    for b in range(B)], axis=0)
    # un-flip the per-chunk row reversal (device writes chunk rows reversed)
    out = out.reshape(B, L // 128, 128, D)[:, :, ::-1, :].reshape(B, L, D)
    return np.ascontiguousarray(out, dtype=np.float32)



# revision 14
# speedup vs baseline: 1463.7601x; 1.0113x over previous
"""Trainium2 Bass kernel for nn_DeChunkLayer (H-Net dechunk: EMA over chunks +
broadcast back to token positions).

Formulation: instead of (argsort -> EMA over M -> gather back to L), run ONE
first-order linear recurrence over the L-length token axis:
    a_l = mask_l ? (1 - p_l) : 1
    b_l = mask_l ? p_l * x[pbi_l] : 0        (pbi = cumsum(mask) - 1)
    H_l = a_l * H_{l-1} + b_l
Then out[l] = H_l exactly (at the m-th boundary H becomes ema[m]; in between it
holds). No argsort/compaction and no output gather; the only data-dependent
movement is the row gather x[pbi_l], done with the HW-accelerated dma_gather.

Per chunk of 128 positions the recurrence is solved with matmuls:
    out[i] = sum_{j<=i} exp(S_i - S_j + ln s_j) * x[pbi_j]  +  exp(S_i) * H_prev
(S = within-chunk inclusive cumsum of log a; ln s folds the s_j scale and the
boundary mask into the exp bias). The chunk is laid out REVERSED on the output
partitions (row 0 = chunk end), which makes the inter-chunk state h_c EQUAL
the chunk's final output row 0: the carry rank-1 matmul for chunk m reads its
rhs directly from row 0 of chunk m-1's staged output tile in SBUF -- there is
no separate H-chain op at all.

Performance structure (sim: 320us baseline -> 117us):
 - bf16 data path (tolerance is 2e-2): x ships bf16 from the host, the gather
   moves 2KB rows, the Exp activation emits bf16 weights, and the output is
   written bf16 and upconverted on the host. fp8 x was tried and FAILS the
   tolerance (e3m4's 3.1% relerr lands on max-magnitude outputs).
 - The triangular -inf mask is preloaded into PSUM via a transpose-matmul of
   the (symmetric) mask matrix, so weight build is PE+ACT only.
 - All 64 carry rows (esr) are computed in ONE batched setup exp and bounced
   through DRAM into a single partition-0 row (matmul lhsT must sit at
   partition base 0).
 - Software pipelining: weights for chunk c+1 and the gather for chunk c+PF
   are issued before chunk c's dependent tail; chunk m's carry+copies+DMA are
   issued one iteration late so no engine queue head-of-line blocks the
   serial carry[q] -> copy[q] -> carry[q] path. Carries and PSUM->SBUF copies
   run at aligned quarter granularity; DVE and ACT split the quarters
   ~950ns/chunk each, with output DMA flushed per half.
 - GPSIMD (Pool) cannot touch PSUM; it only runs gather descriptor gen,
   batched GB=2 chunks per call (994ns fixed SWDGE overhead per call) and
   interleaved finely with the output DMAs on the shared DMA engines.
 - Setup: the gather-index pipeline (wrapped-16 cumsum -> int16 idx) runs
   first, in fp16 (values <= 2048 exact), from one packed input DMA, so the
   first gather fires ~6us in while the weights path streams in behind.

Sharded over batch: core b handles batch row b.
"""

import numpy as np

import concourse.bass as bass
import concourse.tile as tile
from concourse import bacc, mybir
from concourse.bass import IndirectOffsetOnAxis

F32 = mybir.dt.float32
BF16 = mybir.dt.bfloat16
FP16 = mybir.dt.float16
I16 = mybir.dt.int16
I32 = mybir.dt.int32
AX = mybir.AluOpType
ACT = mybir.ActivationFunctionType

# Problem constants (hardcoded per contract)
B, L, D, M = 8, 8192, 1024, 2048
EPS = 1e-4
N_CORES = 8
NEG_BIG = -1e30


def build_program(L_=L, D_=D, M_=M, reps=1, gather_mode="antgather"):
    """Build the per-core Bass program. Returns (nc, names dict)."""
    CH = 128                       # chunk length (= matmul K)
    NCH = L_ // CH                 # number of chunks
    NF = L_ // 16                  # wrapped-16 index columns
    assert NCH * CH == L_
    NSPL = min(512, D_)            # matmul free-dim split (psum bank = 512 f32)
    NH = D_ // NSPL

    from contextlib import ExitStack

    nc = bacc.Bacc(None, target_bir_lowering=False, debug=False)
    with tile.TileContext(nc) as tc, ExitStack() as ctx:
        dram = ctx.enter_context(tc.tile_pool(name="dram", bufs=1, space="DRAM"))
        x_d = dram.tile([M_, D_], BF16, kind="ExternalInput")
        # packed setup inputs: one DMA each (HWDGE gen is per-partition-count)
        s16_d = dram.tile([16, 160 + NF], FP16, kind="ExternalInput")
        big_d = dram.tile([128, 384], F32, kind="ExternalInput")
        pm_d = dram.tile([NCH, 2 * CH], F32, kind="ExternalInput")
        out_d = dram.tile([L_, D_], BF16, kind="ExternalOutput")

        setup = ctx.enter_context(tc.tile_pool(name="setup", bufs=1))
        bsp = ctx.enter_context(tc.tile_pool(name="bsp", bufs=2, space="PSUM"))
        xgp = ctx.enter_context(tc.tile_pool(name="xgp", bufs=3))
        ttp = ctx.enter_context(tc.tile_pool(name="ttp", bufs=3))
        osb = ctx.enter_context(tc.tile_pool(name="osb", bufs=10))

        # ---------------- setup ----------------
        # The gather-index pipeline (m16 -> c16 -> idx16) loads and computes
        # FIRST (in fp16: all values <= 2048 are exact) so the first
        # dma_gather starts while the rest of setup streams in behind it.
        s16 = setup.tile([16, 160 + NF], FP16)
        nc.sync.dma_start(out=s16[:], in_=s16_d[:])
        le16 = s16[:, 0:16]
        gt16 = s16[:, 16:32]
        rep16 = s16[:, 32:160]
        m16 = s16[:, 160:160 + NF]
        ones16 = setup.tile([16, 1], FP16)
        nc.vector.memset(ones16[:], 1.0)

        # FAST PATH: the first PF gathers need only the first 32 wrapped
        # columns; compute those in a separate small tile so the first
        # gather fires ~6us earlier (per-partition cumsum is column-local)
        FW = 32
        idx16a = setup.tile([128, FW], I16)
        idx16b = setup.tile([128, NF - FW], I16)
        with tc.tile_pool(name="bsps", bufs=1, space="PSUM") as bsps:
            cm = setup.tile([16, FW], FP16)
            nc.vector.tensor_tensor_scan(
                out=cm[:], data0=ones16[:].to_broadcast([16, FW]),
                data1=m16[:, 0:FW],
                initial=0.0, op0=AX.mult, op1=AX.add)
            mini_ps = bsps.tile([16, FW], F32, tag="bsm")
            nc.tensor.matmul(out=mini_ps[0:16, 0:FW], lhsT=le16,
                             rhs=cm[:], start=True, stop=False,
                             skip_group_check=True)
            nc.tensor.matmul(out=mini_ps[0:16, 1:FW], lhsT=gt16,
                             rhs=cm[0:16, 0:FW - 1],
                             start=False, stop=True, skip_group_check=True)
            pbi_m = setup.tile([16, FW], FP16)
            nc.vector.tensor_scalar_add(out=pbi_m[:],
                                        in0=mini_ps[0:16, 0:FW],
                                        scalar1=-1.0)
            minir_ps = bsps.tile([128, FW], F32, tag="bsmr")
            nc.tensor.matmul(out=minir_ps[0:128, 0:FW], lhsT=rep16,
                             rhs=pbi_m[:], start=True, stop=True)
            nc.vector.tensor_copy(out=idx16a[:],
                                  in_=minir_ps[0:128, 0:FW])

            # full-width path for the remaining columns
            c16 = setup.tile([16, NF], FP16)
            nc.vector.tensor_tensor_scan(
                out=c16[:], data0=ones16[:].to_broadcast([16, NF]),
                data1=m16,
                initial=0.0, op0=AX.mult, op1=AX.add)
            pbi16_ps = bsps.tile([16, NF], F32, tag="bs16")
            nc.tensor.matmul(out=pbi16_ps[0:16, FW:NF], lhsT=le16,
                             rhs=c16[:, FW:NF], start=True, stop=False,
                             skip_group_check=True)
            nc.tensor.matmul(out=pbi16_ps[0:16, FW:NF], lhsT=gt16,
                             rhs=c16[0:16, FW - 1:NF - 1],
                             start=False, stop=True, skip_group_check=True)
            pbi16 = setup.tile([16, NF], FP16)
            nc.vector.tensor_scalar_add(out=pbi16[0:16, FW:NF],
                                        in0=pbi16_ps[0:16, FW:NF],
                                        scalar1=-1.0)
            # replicate the 16 wrapped index rows to all 8 gpsimd core
            # slots with one fp16 matmul (values <= 2047: exact)
            idxrep_ps = bsps.tile([128, NF], F32, tag="bs16r")
            nc.tensor.matmul(out=idxrep_ps[0:128, FW:NF], lhsT=rep16,
                             rhs=pbi16[:, FW:NF], start=True, stop=True)
            nc.vector.tensor_copy(out=idx16b[:],
                                  in_=idxrep_ps[0:128, FW:NF])

        big = setup.tile([128, 384], F32)
        nc.sync.dma_start(out=big[:], in_=big_d[:])
        ident = big[:, 0:128]
        rev128 = big[:, 128:256]
        mnegr = big[:, 256:384]

        pm = setup.tile([NCH, 2 * CH], F32)
        nc.sync.dma_start(out=pm[:], in_=pm_d[:])
        praw = pm[:, 0:CH]
        mk = pm[:, CH:2 * CH]

        ones_r = setup.tile([NCH, CH], F32)
        nc.vector.memset(ones_r[:], 1.0)

        pc = setup.tile([NCH, CH], F32)
        nc.vector.tensor_scalar(out=pc[:], in0=praw, scalar1=EPS,
                                scalar2=1.0 - EPS, op0=AX.max, op1=AX.min)
        q = setup.tile([NCH, CH], F32)
        nc.vector.tensor_scalar(out=q[:], in0=pc[:], scalar1=-1.0,
                                scalar2=1.0, op0=AX.mult, op1=AX.add)
        lnq = setup.tile([NCH, CH], F32)
        nc.scalar.activation(out=lnq[:], in_=q[:], func=ACT.Ln)
        loga = setup.tile([NCH, CH], F32)
        nc.vector.tensor_tensor(out=loga[:], in0=lnq[:], in1=mk, op=AX.mult)

        # ln(s) with the mask folded in: ln(p) where mask else -1e30
        lnp = setup.tile([NCH, CH], F32)
        nc.scalar.activation(out=lnp[:], in_=pc[:], func=ACT.Ln)
        lnp_m = setup.tile([NCH, CH], F32)
        nc.vector.tensor_tensor(out=lnp_m[:], in0=lnp[:], in1=mk, op=AX.mult)
        mgate = setup.tile([NCH, CH], F32)
        nc.vector.tensor_scalar(out=mgate[:], in0=mk, scalar1=-NEG_BIG,
                                scalar2=NEG_BIG, op0=AX.mult, op1=AX.add)
        lns = setup.tile([NCH, CH], F32)
        nc.vector.tensor_tensor(out=lns[:], in0=lnp_m[:], in1=mgate[:],
                                op=AX.add)

        # within-chunk inclusive cumsum of log(a) (along free dim)
        s_i = setup.tile([NCH, CH], F32)
        nc.vector.tensor_tensor_scan(out=s_i[:], data0=ones_r[:], data1=loga[:],
                                     initial=0.0, op0=AX.mult, op1=AX.add)

        # indices in [CH, NCH] int32 layout (for the indirect_dma fallback)
        c_i = setup.tile([NCH, CH], F32)
        nc.vector.tensor_tensor_scan(out=c_i[:], data0=ones_r[:], data1=mk[:],
                                     initial=0.0, op0=AX.mult, op1=AX.add)
        cnt_colT = bsp.tile([128, 128], F32, tag="bs")
        nc.tensor.transpose(out=cnt_colT[0:1, 0:NCH], in_=c_i[:, CH - 1:CH],
                            identity=big[0:NCH, 0:NCH])
        cnt_row = setup.tile([1, NCH], F32)
        nc.vector.tensor_copy(out=cnt_row[:], in_=cnt_colT[0:1, 0:NCH])
        ones1 = setup.tile([1, 128], F32)
        nc.vector.memset(ones1[:], 1.0)
        cum_row = setup.tile([1, NCH], F32)
        nc.vector.tensor_tensor_scan(out=cum_row[:], data0=ones1[0:1, 0:NCH],
                                     data1=cnt_row[:], initial=0.0,
                                     op0=AX.mult, op1=AX.add)
        bases_row = setup.tile([1, NCH], F32)
        nc.vector.memset(bases_row[:], 0.0)
        nc.vector.tensor_copy(out=bases_row[0:1, 1:NCH],
                              in_=cum_row[0:1, 0:NCH - 1])
        bases_colT = bsp.tile([128, 128], F32, tag="bs")
        nc.tensor.transpose(out=bases_colT[0:NCH, 0:1], in_=bases_row[:],
                            identity=big[0:1, 0:1])
        bases_col = setup.tile([NCH, 1], F32)
        nc.vector.tensor_copy(out=bases_col[:], in_=bases_colT[0:NCH, 0:1])
        pbi_i = setup.tile([NCH, CH], F32)
        nc.vector.tensor_scalar(out=pbi_i[:], in0=c_i[:], scalar1=bases_col[:],
                                scalar2=-1.0, op0=AX.add, op1=AX.add)
        pbiT_ps = bsp.tile([128, 128], F32, tag="bs")
        nc.tensor.transpose(out=pbiT_ps[0:CH, 0:NCH], in_=pbi_i[:],
                            identity=big[0:NCH, 0:NCH])
        idxT = setup.tile([CH, NCH], I32)
        nc.vector.tensor_copy(out=idxT[:], in_=pbiT_ps[0:CH, 0:NCH])

        # transposed per-chunk columns: S, and bias = ln(s) - S
        ST_ps = bsp.tile([128, 128], F32, tag="bs")
        nc.tensor.transpose(out=ST_ps[0:CH, 0:NCH], in_=s_i[:],
                            identity=big[0:NCH, 0:NCH])
        ST = setup.tile([CH, NCH], F32)
        nc.vector.tensor_copy(out=ST[:], in_=ST_ps[0:CH, 0:NCH])
        lnsT_ps = bsp.tile([128, 128], F32, tag="bs")
        nc.tensor.transpose(out=lnsT_ps[0:CH, 0:NCH], in_=lns[:],
                            identity=big[0:NCH, 0:NCH])
        nbT = setup.tile([CH, NCH], F32)
        nc.vector.tensor_tensor(out=nbT[:], in0=lnsT_ps[0:CH, 0:NCH],
                                in1=ST[:], op=AX.subtract)

        # all carry rows at once: esr_all[c, i'] = exp(S^c_{127-i'})
        # (ST already holds s_i transposed; multiply by rev to flip free dim).
        # Matmul lhsT must sit at partition base 0, so bounce the [NCH, CH]
        # tile through DRAM and reload it as one [1, NCH*CH] partition-0 row.
        srev_ps = bsp.tile([128, 128], F32, tag="bs")
        nc.tensor.matmul(out=srev_ps[0:NCH, 0:CH], lhsT=ST[:],
                         rhs=rev128, is_transpose=True,
                         start=True, stop=True)
        esr_all = setup.tile([NCH, CH], BF16)
        nc.scalar.activation(out=esr_all[:], in_=srev_ps[0:NCH, 0:CH],
                             func=ACT.Exp)
        esr_d = dram.tile([NCH, CH], BF16)
        nc.sync.dma_start(out=esr_d[:], in_=esr_all[:])
        esr_row = setup.tile([1, NCH * CH], BF16)
        nc.sync.dma_start(out=esr_row[:],
                          in_=esr_d[:].rearrange("a b -> (a b)"))

        # ---------------- main loop ----------------
        # created after the setup's scoped psum pool is released: 2 (bs)
        # + 3*2 (outp) = 8 banks
        outa = ctx.enter_context(tc.tile_pool(name="outa", bufs=3,
                                              space="PSUM"))
        outb = ctx.enter_context(tc.tile_pool(name="outb", bufs=3,
                                              space="PSUM"))
        GB = 2                      # chunks per batched gather call

        for _rep in range(reps):
            xg_tiles = {}
            wt_tiles = {}

            def issue_gather(c0):
                """One SWDGE call gathers GB chunks (994ns fixed overhead per
                call); group g of the out tile = chunk c0+g."""
                if gather_mode == "antgather":
                    xgb = xgp.tile([CH, GB, D_], BF16, tag="xg")
                    if 8 * (c0 + GB) <= FW:
                        idxs = idx16a[:, 8 * c0:8 * (c0 + GB)]
                    else:
                        idxs = idx16b[:, 8 * c0 - FW:8 * (c0 + GB) - FW]
                    nc.gpsimd.dma_gather(
                        out_ap=xgb[:],
                        in_ap=x_d[:],
                        idxs_ap=idxs,
                        num_idxs=CH * GB, num_idxs_reg=CH * GB,
                        elem_size=D_)
                    for g in range(GB):
                        xg_tiles[c0 + g] = (xgb, g)
                elif gather_mode == "indirect":
                    for g in range(GB):
                        xgt = xgp.tile([CH, 1, D_], BF16, tag="xgs")
                        nc.gpsimd.indirect_dma_start(
                            out=xgt[:, 0, :], out_offset=None, in_=x_d[:],
                            in_offset=IndirectOffsetOnAxis(
                                ap=idxT[:, c0 + g:c0 + g + 1], axis=0))
                        xg_tiles[c0 + g] = (xgt, 0)
                else:
                    raise ValueError(gather_mode)

            def issue_weights(c):
                """PSUM <- mnegr (symmetric triangular -inf mask), then
                accumulate Sbc[j, i'] = S_{127-i'} (PE transpose of the
                free-broadcast S column against the anti-diagonal perm);
                then weights ttm[j, i'] = exp(S_{127-i'} - S_j + ln s_j)."""
                sbc = bsp.tile([128, 128], F32, tag="bs")
                nc.tensor.matmul(out=sbc[0:CH, 0:CH], lhsT=mnegr,
                                 rhs=ident, is_transpose=True,
                                 start=True, stop=False, skip_group_check=True)
                nc.tensor.matmul(out=sbc[0:CH, 0:CH],
                                 lhsT=ST[:, c:c + 1].to_broadcast([CH, CH]),
                                 rhs=rev128, is_transpose=True,
                                 start=False, stop=True, skip_group_check=True)
                ttm = ttp.tile([CH, CH], BF16, tag="ttm")
                nc.scalar.activation(out=ttm[:], in_=sbc[0:CH, 0:CH],
                                     func=ACT.Exp, bias=nbT[:, c:c + 1])
                wt_tiles[c] = ttm

            op_tiles = {}
            o_tiles = {}

            NQ = D_ // 4

            def finish_chunk(m):
                """Carry accumulation + output for chunk m, issued one
                iteration late. The carry rhs is row 0 of the PREVIOUS chunk's
                staged output: with reversed rows, out row 0 = position 127 =
                H at chunk end including its own carry = h_{m-1}, already in
                SBUF bf16 -- no separate H-chain op needed. Carry matmuls and
                PSUM->SBUF copies run at aligned QUARTER granularity: each
                quarter is an independent serial sub-path
                (carry[qk] -> copy[qk] -> next carry[qk]) whose copy engine
                (DVE for even quarters, ACT for odd) never gates another
                quarter's path."""
                opa, opb = op_tiles.pop(m)
                o_sb = osb.tile([CH, D_], BF16, tag="osb")
                o_tiles[m] = o_sb
                prev = o_tiles.pop(m - 1, None)
                # q0/q1 (outa) on DVE, q2/q3 (outb) on ACT: the two copy
                # lanes never couple through a shared psum tile's WAR, so an
                # ACT lag (it also runs the weight exps) can't stall the
                # DVE-side carry chain or the next chunk's main matmul.
                for k in range(4):
                    qk = slice(k * NQ, (k + 1) * NQ)
                    half = opa if k < 2 else opb
                    hq = slice((k % 2) * NQ, (k % 2 + 1) * NQ)
                    if m > 0:
                        nc.tensor.matmul(out=half[0:CH, hq],
                                         lhsT=esr_row[0:1,
                                                      m * CH:(m + 1) * CH],
                                         rhs=prev[0:1, qk],
                                         start=False, stop=True,
                                         skip_group_check=True)
                    if k < 2:
                        nc.vector.tensor_copy(out=o_sb[:, qk],
                                              in_=half[0:CH, hq])
                    else:
                        nc.scalar.activation(out=o_sb[:, qk],
                                             in_=half[0:CH, hq],
                                             func=ACT.Copy)
                    if k == 3:
                        nc.sync.dma_start(
                            out=out_d[m * CH:(m + 1) * CH, :],
                            in_=o_sb[:, :])

            # software-pipelined prologue (gathers prefetched PF deep)
            PF = 4
            for g0 in range(0, min(PF, NCH), GB):
                issue_gather(g0)
            issue_weights(0)

            for c in range(NCH):
                # gather prefetch (Pool only does gather gen now)
                if c % GB == 0 and c + PF < NCH:
                    issue_gather(c + PF)
                # previous chunk's carry + output FIRST: its ops head every
                # engine queue, so the serial carry->copy->carry path never
                # waits behind this iteration's prefetch work
                if c > 0:
                    finish_chunk(c - 1)
                if c + 1 < NCH:
                    issue_weights(c + 1)
                ttm = wt_tiles.pop(c)
                xg_t, xg_g = xg_tiles.pop(c)

                # main matmul (reversed rows; row 0 = chunk end, sans carry)
                opa = outa.tile([128, NSPL], F32, tag="opa")
                opb = outb.tile([128, NSPL], F32, tag="opb")
                op_tiles[c] = (opa, opb)
                for h, half in enumerate((opa, opb)):
                    sl = slice(h * NSPL, (h + 1) * NSPL)
                    nc.tensor.matmul(out=half[0:CH, 0:NSPL], lhsT=ttm[:],
                                     rhs=xg_t[:, xg_g, sl],
                                     start=True, stop=True)


            finish_chunk(NCH - 1)

    nc.compile()
    names = dict(x=x_d.name, s16=s16_d.name, big=big_d.name, pm=pm_d.name,
                 out=out_d.name)
    return nc, names


def make_consts():
    ident = np.eye(128, dtype=np.float32)
    rev = np.eye(128, dtype=np.float32)[::-1].copy()
    jj = np.arange(128)
    # reversed triangular mask: out-row i' holds position (127 - i')
    mnegr = np.where(jj[:, None] > 127 - jj[None, :], NEG_BIG, 0.0).astype(
        np.float32)
    p16 = np.arange(16)
    le16 = (p16[:, None] <= p16[None, :]).astype(np.float16)
    gt16 = (p16[:, None] > p16[None, :]).astype(np.float16)
    rep16 = (p16[:, None] == (np.arange(128) % 16)[None, :]).astype(
        np.float16)
    big = np.concatenate([ident, rev, mnegr], axis=1)
    return dict(big=big, le16=le16, gt16=gt16, rep16=rep16)


_CACHE = {}


def _get_program():
    if "prog" not in _CACHE:
        _CACHE["prog"] = build_program()
    return _CACHE["prog"]


def per_core_inputs(names, hidden_b, bprob_b, mask_b, L_=L):
    import ml_dtypes

    NCH = L_ // 128
    NF = L_ // 16
    cs = make_consts()
    mf = mask_b.astype(np.float32)
    s16 = np.concatenate(
        [cs["le16"], cs["gt16"], cs["rep16"],
         np.ascontiguousarray(mf.reshape(NF, 16).T).astype(np.float16)],
        axis=1)
    pm = np.concatenate([np.ascontiguousarray(
        bprob_b[:, 1].reshape(NCH, 128)), mf.reshape(NCH, 128)], axis=1)
    return {
        names["x"]: np.ascontiguousarray(hidden_b).astype(ml_dtypes.bfloat16),
        names["s16"]: np.ascontiguousarray(s16),
        names["big"]: np.ascontiguousarray(cs["big"]),
        names["pm"]: np.ascontiguousarray(pm),
    }


def kernel(hidden_states, boundary_prob, boundary_mask):
    from concourse import bass_utils

    nc, names = _get_program()

    hidden_states = np.asarray(hidden_states, dtype=np.float32)
    boundary_prob = np.asarray(boundary_prob, dtype=np.float32)
    boundary_mask = np.asarray(boundary_mask)

    in_maps = [per_core_inputs(names, hidden_states[b], boundary_prob[b],
                               boundary_mask[b]) for b in range(B)]
    res = bass_utils.run_bass_kernel_spmd(nc, in_maps,
                                          core_ids=list(range(N_CORES)))
    out = np.stack([np.asarray(res.results[b][names["out"]]).astype(np.float32)
                    for b in range(B)], axis=0)
    # un-flip the per-chunk row reversal (device writes chunk rows reversed)
    out = out.reshape(B, L // 128, 128, D)[:, :, ::-1, :].reshape(B, L, D)
    return np.ascontiguousarray(out, dtype=np.float32)

